# revision 4
# baseline (speedup 1.0000x reference)
"""Trainium2 Bass kernel for nn_BlockALiBi (dense transformer block with ALiBi
attention that also returns the full attention-weight tensor).

Reference computation (B=4, T=2048, D=512, H=8, HS=64):
    hn = LN(x; g1, be1)
    q,k,v = per-head projections of hn (no bias)
    scores = q @ k^T / sqrt(HS) + alibi  (alibi[h, s] = -slope_h * s)
    attn = softmax(scores, axis=-1)
    x = x + (concat_heads(attn @ v)) @ Wp + bp
    x = x + relu(LN(x; g2, be2) @ W1 + b1) @ W2 + b2
    returns (x, attn transposed to [H,B,T,T])

Sharding: 8 cores; core c owns (batch b = c//2, query half th = c%2, 1024
query rows).  Each core computes the block for its query rows and emits its
[H, 1024, T] slice of the attention weights.  No collectives.

Key algebraic fact exploited: the ALiBi slopes are 0.5*(h+1) and |scores| is
provably bounded (Cauchy-Schwarz with spectral norms computed host-side), so
for key positions s >= SK (512 here) the reference softmax's exp(x - max) has
argument < -104 and underflows to an exact 0.0 in fp32.  The kernel computes
attention over the first SK keys only and writes exact zeros elsewhere; the
bound is validated at runtime against the actual weights (falls back to a
larger SK if ever violated).
"""

import numpy as np

B, T, D, H = 4, 2048, 512, 8
HS = D // H           # 64
FD = 4 * D            # 2048 ffn hidden
NC = 8                # cores
TQ = T // 2           # query rows per core
EPS = 1e-5

_BUILT = {}           # (SK, explicit_zeros) -> finalized Bass object


def _build(SK, explicit_zeros=True):
    """Build and finalize the per-core Bass kernel for SK kept key positions."""
    import concourse.bass as bass
    import concourse.tile as tile
    from concourse import bacc, mybir

    F32 = mybir.dt.float32
    AF = mybir.ActivationFunctionType
    OP = mybir.AluOpType

    SKT = SK // 128           # s-tiles of 128 (phase B / v)
    SKC = SK // 512           # s-chunks of 512 (phase A scores)
    NLN = (SK + TQ) // 128    # LN1 token tiles (keys then queries)
    ZW = T - SK               # zero-filled tail width per attn row

    nc = bacc.Bacc("TRN2", target_bir_lowering=False)

    # ---- I/O ----
    x_ln = nc.dram_tensor("x_ln", [SK + TQ, D], F32, kind="ExternalInput")
    x_res = nc.dram_tensor("x_res", [TQ, D], F32, kind="ExternalInput")
    wq_d = nc.dram_tensor("wq", [D, D], F32, kind="ExternalInput")
    wk_d = nc.dram_tensor("wk", [D, D], F32, kind="ExternalInput")
    wv_d = nc.dram_tensor("wv", [D, D], F32, kind="ExternalInput")
    wp_d = nc.dram_tensor("wp", [D, D], F32, kind="ExternalInput")
    w1_d = nc.dram_tensor("w1", [D, FD], F32, kind="ExternalInput")
    w2_d = nc.dram_tensor("w2", [FD, D], F32, kind="ExternalInput")
    qkb_d = nc.dram_tensor("qkbias", [128, 8], F32, kind="ExternalInput")
    bv_d = nc.dram_tensor("bv_row", [1, D], F32, kind="ExternalInput")
    b1c_d = nc.dram_tensor("b1col", [128, FD // 128], F32, kind="ExternalInput")
    b2_d = nc.dram_tensor("b2_row", [1, D], F32, kind="ExternalInput")
    alibi_d = nc.dram_tensor("alibi", [H, SK], F32, kind="ExternalInput")
    ident_d = nc.dram_tensor("ident", [128, 128], F32, kind="ExternalInput")

    attn_o = nc.dram_tensor("attn", [H, TQ, T], F32, kind="ExternalOutput")
    xout_o = nc.dram_tensor("xout", [TQ, D], F32, kind="ExternalOutput")

    def bcast_ap(dram_handle, p=128):
        # partition-replicating DMA source (step 0 over partitions)
        ap = dram_handle[:, :]
        return bass.AP(tensor=ap.tensor, offset=ap.offset,
                       ap=[[0, p]] + list(ap.ap))

    with tile.TileContext(nc) as tc:
        with tc.tile_pool(name="consts", bufs=1) as consts, \
             tc.tile_pool(name="outT_p", bufs=1) as outT_p, \
             tc.tile_pool(name="x2_p", bufs=8) as x2_p:

            ident = consts.tile([128, 128], F32)
            nc.gpsimd.dma_start(out=ident[:], in_=ident_d[:, :])
            qkb = consts.tile([128, 8], F32)
            nc.gpsimd.dma_start(out=qkb[:], in_=qkb_d[:, :])
            bv_bc = consts.tile([128, D], F32)
            nc.gpsimd.dma_start(out=bv_bc[:], in_=bcast_ap(bv_d))
            b2_bc = consts.tile([128, D], F32)
            nc.gpsimd.dma_start(out=b2_bc[:], in_=bcast_ap(b2_d))
            b1c = consts.tile([128, FD // 128], F32)
            nc.gpsimd.dma_start(out=b1c[:], in_=b1c_d[:, :])
            eps_t = consts.tile([128, 1], F32)
            nc.vector.memset(eps_t, EPS)
            ones1 = consts.tile([1, 64], F32)
            nc.vector.memset(ones1, 1.0)
            if explicit_zeros:
                zt = consts.tile([128, ZW], F32)
                nc.vector.memset(zt, 0.0)

            outT = outT_p.tile([128, 4, TQ], F32)   # concat head outputs ^T

            # =========== stages 1-3 scope ===========
            with tc.tile_pool(name="s13", bufs=1) as s13, \
                 tc.tile_pool(name="xt_p", bufs=3) as xt_p, \
                 tc.tile_pool(name="st_p", bufs=6) as st_p, \
                 tc.tile_pool(name="cn_p", bufs=3) as cn_p, \
                 tc.tile_pool(name="e_p", bufs=3) as e_p, \
                 tc.tile_pool(name="et_p", bufs=4) as et_p, \
                 tc.tile_pool(name="sm_p", bufs=8) as sm_p:

                hnT = s13.tile([128, 4, SK + TQ], F32)
                wq_sb = s13.tile([128, 4, D], F32)
                wk_sb = s13.tile([128, 4, D], F32)
                wv_sb = s13.tile([128, 4, D], F32)
                nc.gpsimd.dma_start(out=wq_sb[:], in_=wq_d[:, :].rearrange("(o p) n -> p o n", p=128))
                nc.gpsimd.dma_start(out=wk_sb[:], in_=wk_d[:, :].rearrange("(o p) n -> p o n", p=128))
                nc.gpsimd.dma_start(out=wv_sb[:], in_=wv_d[:, :].rearrange("(o p) n -> p o n", p=128))

                qT = s13.tile([65, 8, TQ], F32)
                kT = s13.tile([65, 8, SK], F32)
                vA = s13.tile([128, SKT, 8, 65], F32)
                nc.vector.memset(qT[64:65, :, :], 1.0)
                nc.gpsimd.dma_start(out=kT[64:65, :, :], in_=alibi_d[:, :])
                nc.vector.memset(vA[:, :, :, 64:65], 1.0)

                # ---- stages 1+2 (own psum scope) ----
                with tc.tile_pool(name="ps_t", bufs=2, space="PSUM") as ps_t, \
                     tc.tile_pool(name="ps_qkv", bufs=3, space="PSUM") as ps_qkv:

                    # LN1 + transpose into hnT
                    for i in range(NLN):
                        xt = xt_p.tile([128, D], F32)
                        nc.sync.dma_start(out=xt[:], in_=x_ln[i * 128:(i + 1) * 128, :])
                        stats = st_p.tile([128, 6], F32)
                        nc.vector.bn_stats(out=stats[:], in_=xt[:])
                        mv = st_p.tile([128, 2], F32)
                        nc.vector.bn_aggr(out=mv[:], in_=stats[:])
                        rstd = st_p.tile([128, 1], F32)
                        nc.scalar.activation(out=rstd[:], in_=mv[:, 1:2], func=AF.Sqrt,
                                             bias=eps_t[:], scale=1.0)
                        nc.vector.reciprocal(rstd[:], rstd[:])
                        cn = cn_p.tile([128, D], F32)
                        nc.vector.tensor_scalar(out=cn[:], in0=xt[:], scalar1=mv[:, 0:1],
                                                scalar2=rstd[:], op0=OP.subtract, op1=OP.mult)
                        for j in range(4):
                            pt = ps_t.tile([128, 128], F32)
                            nc.tensor.transpose(pt[:], cn[:, j * 128:(j + 1) * 128], ident[:])
                            nc.scalar.copy(out=hnT[:, j, i * 128:(i + 1) * 128], in_=pt[:])

                    # QKV
                    for h in range(H):
                        for c in range(TQ // 512):
                            pq = ps_qkv.tile([64, 512], F32, tag="pqk")
                            for j in range(4):
                                nc.tensor.matmul(pq[:], wq_sb[:, j, h * 64:(h + 1) * 64],
                                                 hnT[:, j, SK + c * 512: SK + (c + 1) * 512],
                                                 start=(j == 0), stop=(j == 3))
                            nc.vector.tensor_scalar(out=qT[0:64, h, c * 512:(c + 1) * 512],
                                                    in0=pq[:], scalar1=qkb[0:64, h:h + 1],
                                                    scalar2=None, op0=OP.add)
                        for c in range(SKC):
                            pk = ps_qkv.tile([64, 512], F32, tag="pqk")
                            for j in range(4):
                                nc.tensor.matmul(pk[:], wk_sb[:, j, h * 64:(h + 1) * 64],
                                                 hnT[:, j, c * 512:(c + 1) * 512],
                                                 start=(j == 0), stop=(j == 3))
                            nc.vector.tensor_scalar(out=kT[0:64, h, c * 512:(c + 1) * 512],
                                                    in0=pk[:], scalar1=qkb[64:128, h:h + 1],
                                                    scalar2=None, op0=OP.add)
                    for i in range(SKT):
                        pv = ps_qkv.tile([128, D], F32, tag="pv")
                        for j in range(4):
                            nc.tensor.matmul(pv[:], hnT[:, j, i * 128:(i + 1) * 128],
                                             wv_sb[:, j, :], start=(j == 0), stop=(j == 3))
                        nc.vector.tensor_tensor(
                            vA[:, i, :, 0:64],
                            pv[:].rearrange("p (h k) -> p h k", h=H),
                            bv_bc[:].rearrange("p (h k) -> p h k", h=H),
                            OP.add)

                # ---- stage 3: attention (own psum scope) ----
                with tc.tile_pool(name="ps_a", bufs=2, space="PSUM") as ps_a, \
                     tc.tile_pool(name="ps_b", bufs=2, space="PSUM") as ps_b, \
                     tc.tile_pool(name="ps_o", bufs=2, space="PSUM") as ps_o, \
                     tc.tile_pool(name="ps_bc", bufs=2, space="PSUM") as ps_bc:
                    for h in range(H):
                        # phase B: out^T_h = v~^T @ exp(scores^T); den lands in row 64
                        for c in range(TQ // 512):
                            po = ps_o.tile([65, 512], F32)
                            for si in range(SKT):
                                pb = ps_b.tile([128, 512], F32)
                                nc.tensor.matmul(pb[:], kT[:, h, si * 128:(si + 1) * 128],
                                                 qT[:, h, c * 512:(c + 1) * 512])
                                et = et_p.tile([128, 512], F32)
                                nc.scalar.activation(out=et[:], in_=pb[:], func=AF.Exp)
                                nc.tensor.matmul(po[:], vA[:, si, h, :], et[:],
                                                 start=(si == 0), stop=(si == SKT - 1))
                            rr = sm_p.tile([1, 512], F32, tag="rr")
                            nc.vector.reciprocal(rr[:], po[64:65, :])
                            pbc = ps_bc.tile([64, 512], F32)
                            nc.tensor.matmul(pbc[:], ones1[:], rr[:])
                            rbc = et_p.tile([64, 512], F32, tag="rbc")
                            nc.vector.tensor_copy(rbc[:], pbc[:])
                            nc.vector.tensor_tensor(
                                outT[(h % 2) * 64:(h % 2) * 64 + 64, h // 2,
                                     c * 512:(c + 1) * 512],
                                po[0:64, :], rbc[:], OP.mult)
                        # phase A: attention-weight rows for the big output
                        for tq in range(TQ // 128):
                            E = e_p.tile([128, SK], F32)
                            den = sm_p.tile([128, 1], F32, tag="den")
                            for c in range(SKC):
                                pa = ps_a.tile([128, 512], F32)
                                nc.tensor.matmul(pa[:], qT[:, h, tq * 128:(tq + 1) * 128],
                                                 kT[:, h, c * 512:(c + 1) * 512])
                                if SKC == 1:
                                    nc.scalar.activation(out=E[:, c * 512:(c + 1) * 512],
                                                         in_=pa[:], func=AF.Exp,
                                                         accum_out=den[:])
                                else:
                                    dpart = sm_p.tile([128, 1], F32, tag="dpart")
                                    nc.scalar.activation(out=E[:, c * 512:(c + 1) * 512],
                                                         in_=pa[:], func=AF.Exp,
                                                         accum_out=dpart[:])
                                    if c == 0:
                                        nc.vector.tensor_copy(den[:], dpart[:])
                                    else:
                                        nc.vector.tensor_add(out=den[:], in0=den[:],
                                                             in1=dpart[:])
                            r = sm_p.tile([128, 1], F32, tag="r")
                            nc.vector.reciprocal(r[:], den[:])
                            nc.vector.tensor_scalar_mul(E[:], E[:], r[:])
                            nc.sync.dma_start(out=attn_o[h, tq * 128:(tq + 1) * 128, 0:SK],
                                              in_=E[:])
                            if explicit_zeros:
                                nc.scalar.dma_start(
                                    out=attn_o[h, tq * 128:(tq + 1) * 128, SK:T],
                                    in_=zt[:])

            # =========== stages 4-5 scope ===========
            with tc.tile_pool(name="s45", bufs=1) as s45, \
                 tc.tile_pool(name="xr_p", bufs=3) as xr_p, \
                 tc.tile_pool(name="st2_p", bufs=6) as st2_p, \
                 tc.tile_pool(name="cn2_p", bufs=3) as cn2_p, \
                 tc.tile_pool(name="f1_p", bufs=1) as f1_p, \
                 tc.tile_pool(name="xo_p", bufs=3) as xo_p, \
                 tc.tile_pool(name="ps_p", bufs=2, space="PSUM") as ps_p, \
                 tc.tile_pool(name="ps_t2", bufs=2, space="PSUM") as ps_t2, \
                 tc.tile_pool(name="ps_f", bufs=2, space="PSUM") as ps_f, \
                 tc.tile_pool(name="ps_2", bufs=2, space="PSUM") as ps_2:

                wp_sb = s45.tile([128, 4, D], F32)
                nc.gpsimd.dma_start(out=wp_sb[:], in_=wp_d[:, :].rearrange("(o p) n -> p o n", p=128))
                w1_sb = s45.tile([128, 4, FD], F32)
                nc.gpsimd.dma_start(out=w1_sb[:], in_=w1_d[:, :].rearrange("(o p) n -> p o n", p=128))
                w2_sb = s45.tile([128, 16, D], F32)
                nc.gpsimd.dma_start(out=w2_sb[:], in_=w2_d[:, :].rearrange("(o p) n -> p o n", p=128))
                h2T = s45.tile([128, 4, TQ], F32)
                x2s = []

                # ---- stage 4: proj + residual + LN2 (+ transpose) ----
                for tq in range(TQ // 128):
                    pp = ps_p.tile([128, D], F32)
                    for j in range(4):
                        nc.tensor.matmul(pp[:], outT[:, j, tq * 128:(tq + 1) * 128],
                                         wp_sb[:, j, :], start=(j == 0), stop=(j == 3))
                    xr = xr_p.tile([128, D], F32)
                    nc.sync.dma_start(out=xr[:], in_=x_res[tq * 128:(tq + 1) * 128, :])
                    x2 = x2_p.tile([128, D], F32, tag="x2")
                    nc.vector.tensor_add(out=x2[:], in0=pp[:], in1=xr[:])
                    x2s.append(x2)
                    stats = st2_p.tile([128, 6], F32)
                    nc.vector.bn_stats(out=stats[:], in_=x2[:])
                    mv = st2_p.tile([128, 2], F32)
                    nc.vector.bn_aggr(out=mv[:], in_=stats[:])
                    rstd = st2_p.tile([128, 1], F32)
                    nc.scalar.activation(out=rstd[:], in_=mv[:, 1:2], func=AF.Sqrt,
                                         bias=eps_t[:], scale=1.0)
                    nc.vector.reciprocal(rstd[:], rstd[:])
                    cn2 = cn2_p.tile([128, D], F32)
                    nc.vector.tensor_scalar(out=cn2[:], in0=x2[:], scalar1=mv[:, 0:1],
                                            scalar2=rstd[:], op0=OP.subtract, op1=OP.mult)
                    for j in range(4):
                        pt = ps_t2.tile([128, 128], F32)
                        nc.tensor.transpose(pt[:], cn2[:, j * 128:(j + 1) * 128], ident[:])
                        nc.scalar.copy(out=h2T[:, j, tq * 128:(tq + 1) * 128], in_=pt[:])

                # ---- stage 5: FFN ----
                for c in range(TQ // 512):
                    f1 = f1_p.tile([128, 16, 512], F32)
                    for m in range(16):
                        pf = ps_f.tile([128, 512], F32)
                        for j in range(4):
                            nc.tensor.matmul(pf[:], w1_sb[:, j, m * 128:(m + 1) * 128],
                                             h2T[:, j, c * 512:(c + 1) * 512],
                                             start=(j == 0), stop=(j == 3))
                        nc.scalar.activation(out=f1[:, m, :], in_=pf[:], func=AF.Relu,
                                             bias=b1c[:, m:m + 1], scale=1.0)
                    for u in range(4):
                        tq = c * 4 + u
                        p2 = ps_2.tile([128, D], F32)
                        for m in range(16):
                            nc.tensor.matmul(p2[:], f1[:, m, u * 128:(u + 1) * 128],
                                             w2_sb[:, m, :], start=(m == 0), stop=(m == 15))
                        xo = xo_p.tile([128, D], F32)
                        nc.vector.tensor_add(out=xo[:], in0=p2[:], in1=x2s[tq][:])
                        nc.vector.tensor_add(out=xo[:], in0=xo[:], in1=b2_bc[:])
                        nc.sync.dma_start(out=xout_o[tq * 128:(tq + 1) * 128, :], in_=xo[:])

    nc.finalize()
    return nc


def _host_prep(x, Wq, Wk, Wv, Wp, bp, W1, b1, W2, b2, g1, be1, g2, be2):
    """Fold LN affine params into projection weights (exact algebra) and
    build per-core input maps."""
    f32 = np.float32
    x = np.asarray(x, f32)
    Wq = np.asarray(Wq, f32); Wk = np.asarray(Wk, f32); Wv = np.asarray(Wv, f32)
    Wp = np.ascontiguousarray(np.asarray(Wp, f32))
    W1 = np.asarray(W1, f32); W2 = np.ascontiguousarray(np.asarray(W2, f32))
    bp = np.asarray(bp, f32); b1 = np.asarray(b1, f32); b2 = np.asarray(b2, f32)
    g1 = np.asarray(g1, f32); be1 = np.asarray(be1, f32)
    g2 = np.asarray(g2, f32); be2 = np.asarray(be2, f32)

    scale = 1.0 / np.sqrt(HS)
    wq_f = np.ascontiguousarray(np.transpose(Wq, (1, 0, 2)).reshape(D, D)
                                * g1[:, None] * scale).astype(f32)
    wk_f = np.ascontiguousarray(np.transpose(Wk, (1, 0, 2)).reshape(D, D)
                                * g1[:, None]).astype(f32)
    wv_f = np.ascontiguousarray(np.transpose(Wv, (1, 0, 2)).reshape(D, D)
                                * g1[:, None]).astype(f32)
    bq = (np.einsum('d,hdk->hk', be1, Wq) * scale).astype(f32)   # [H, HS]
    bk = np.einsum('d,hdk->hk', be1, Wk).astype(f32)
    bv = np.einsum('d,hdk->hk', be1, Wv).reshape(D).astype(f32)
    qkbias = np.zeros((128, 8), f32)
    qkbias[0:64, :] = bq.T
    qkbias[64:128, :] = bk.T

    w1_f = np.ascontiguousarray(W1 * g2[:, None]).astype(f32)
    b1_f = (b1 + be2 @ W1).astype(f32)
    b1col = np.ascontiguousarray(b1_f.reshape(FD // 128, 128).T).astype(f32)

    slopes = (2.0 ** (-8.0 / H)) * np.arange(1, H + 1, dtype=f32)

    # ---- choose SK: reference attn provably exact-zero for s >= SK ----
    sq = np.array([np.linalg.norm(wq_f[:, h * HS:(h + 1) * HS], 2) for h in range(H)])
    sk_ = np.array([np.linalg.norm(wk_f[:, h * HS:(h + 1) * HS], 2) for h in range(H)])
    Bq = np.sqrt(D) * sq + np.linalg.norm(bq, axis=1)
    Bk = np.sqrt(D) * sk_ + np.linalg.norm(bk, axis=1)
    Bh = Bq * Bk      # |scores| <= Bh per head (||cn_row|| <= sqrt(D))
    SK = None
    for cand in (512, 1024):
        if np.all(slopes * cand - 2 * Bh > 105.0):
            SK = cand
            break
    if SK is None:
        raise RuntimeError(f"score bound too large for truncation: Bh={Bh}")

    alibi = np.ascontiguousarray(
        -slopes[:, None] * np.arange(SK, dtype=f32)[None, :]).astype(f32)

    shared = {
        "wq": wq_f, "wk": wk_f, "wv": wv_f, "wp": Wp,
        "w1": w1_f, "w2": W2,
        "qkbias": qkbias, "bv_row": bv.reshape(1, D),
        "b1col": b1col, "b2_row": np.ascontiguousarray(b2.reshape(1, D)),
        "alibi": alibi, "ident": np.eye(128, dtype=f32),
    }
    xbp = x + bp[None, None, :]
    in_maps = []
    for c in range(NC):
        b, th = c // 2, c % 2
        t0 = th * TQ
        m = dict(shared)
        m["x_ln"] = np.ascontiguousarray(
            np.concatenate([x[b, 0:SK], x[b, t0:t0 + TQ]], axis=0))
        m["x_res"] = np.ascontiguousarray(xbp[b, t0:t0 + TQ])
        in_maps.append(m)
    return SK, in_maps


def kernel(x, Wq, Wk, Wv, Wp, bp, W1, b1, W2, b2, g1, be1, g2, be2):
    from concourse.bass_utils import run_bass_kernel_spmd

    SK, in_maps = _host_prep(x, Wq, Wk, Wv, Wp, bp, W1, b1, W2, b2,
                             g1, be1, g2, be2)
    key = (SK, True)
    if key not in _BUILT:
        _BUILT[key] = _build(SK, explicit_zeros=True)
    nc = _BUILT[key]

    res = run_bass_kernel_spmd(nc, in_maps, core_ids=list(range(NC)))

    x_out = np.empty((B, T, D), np.float32)
    attn = np.empty((H, B, T, T), np.float32)
    for c in range(NC):
        b, th = c // 2, c % 2
        t0 = th * TQ
        x_out[b, t0:t0 + TQ] = res.results[c]["xout"]
        attn[:, b, t0:t0 + TQ, :] = res.results[c]["attn"]
    return (x_out, attn)


# revision 8
# speedup vs baseline: 1.1746x; 1.1746x over previous
"""Trainium2 Bass kernel for nn_BlockALiBi (dense transformer block with ALiBi
attention that also returns the full attention-weight tensor).

Reference computation (B=4, T=2048, D=512, H=8, HS=64):
    hn = LN(x; g1, be1)
    q,k,v = per-head projections of hn (no bias)
    scores = q @ k^T / sqrt(HS) + alibi  (alibi[h, s] = -slope_h * s)
    attn = softmax(scores, axis=-1)
    x = x + (concat_heads(attn @ v)) @ Wp + bp
    x = x + relu(LN(x; g2, be2) @ W1 + b1) @ W2 + b2
    returns (x, attn transposed to [H,B,T,T])

Sharding: 8 cores; core c owns (batch b = c//2, query half th = c%2, 1024
query rows).  Each core computes the block for its query rows and emits its
[H, 1024, T] slice of the attention weights.  No collectives.

Key algebraic fact exploited: the ALiBi slopes are 0.5*(h+1) and |scores| is
provably bounded (Cauchy-Schwarz with spectral norms computed host-side), so
for key positions s >= SK (512 here) the reference softmax's exp(x - max) has
argument < -104 and underflows to an exact 0.0 in fp32.  The kernel computes
attention over the first SK keys only and writes exact zeros elsewhere; the
bound is validated at runtime against the actual weights (falls back to a
larger SK if ever violated).
"""

import numpy as np

B, T, D, H = 4, 2048, 512, 8
HS = D // H           # 64
FD = 4 * D            # 2048 ffn hidden
NC = 8                # cores
TQ = T // 2           # query rows per core
EPS = 1e-5

_BUILT = {}           # (SK, explicit_zeros) -> finalized Bass object


def _build(SK, explicit_zeros=True):
    """Build and finalize the per-core Bass kernel for SK kept key positions."""
    import concourse.bass as bass
    import concourse.tile as tile
    from concourse import bacc, mybir

    F32 = mybir.dt.float32
    AF = mybir.ActivationFunctionType
    OP = mybir.AluOpType

    SKT = SK // 128           # s-tiles of 128 (phase B / v)
    SKC = SK // 512           # s-chunks of 512 (phase A scores)
    NLN = (SK + TQ) // 128    # LN1 token tiles (keys then queries)
    ZW = T - SK               # zero-filled tail width per attn row

    nc = bacc.Bacc("TRN2", target_bir_lowering=False)

    # ---- I/O ----
    x_ln = nc.dram_tensor("x_ln", [SK + TQ, D], F32, kind="ExternalInput")
    x_res = nc.dram_tensor("x_res", [TQ, D], F32, kind="ExternalInput")
    wq_d = nc.dram_tensor("wq", [D, D], F32, kind="ExternalInput")
    wk_d = nc.dram_tensor("wk", [D, D], F32, kind="ExternalInput")
    wv_d = nc.dram_tensor("wv", [D, D], F32, kind="ExternalInput")
    wp_d = nc.dram_tensor("wp", [D, D], F32, kind="ExternalInput")
    w1_d = nc.dram_tensor("w1", [D, FD], F32, kind="ExternalInput")
    w2_d = nc.dram_tensor("w2", [FD, D], F32, kind="ExternalInput")
    qkb_d = nc.dram_tensor("qkbias", [128, 8], F32, kind="ExternalInput")
    bv_d = nc.dram_tensor("bv_row", [1, D], F32, kind="ExternalInput")
    b1c_d = nc.dram_tensor("b1col", [128, FD // 128], F32, kind="ExternalInput")
    b2_d = nc.dram_tensor("b2_row", [1, D], F32, kind="ExternalInput")
    alibi_d = nc.dram_tensor("alibi", [H, SK], F32, kind="ExternalInput")
    ident_d = nc.dram_tensor("ident", [128, 128], F32, kind="ExternalInput")

    attn_o = nc.dram_tensor("attn", [H, TQ, T], F32, kind="ExternalOutput")
    xout_o = nc.dram_tensor("xout", [TQ, D], F32, kind="ExternalOutput")

    def bcast_ap(dram_handle, p=128):
        # partition-replicating DMA source (step 0 over partitions)
        ap = dram_handle[:, :]
        return bass.AP(tensor=ap.tensor, offset=ap.offset,
                       ap=[[0, p]] + list(ap.ap))

    with tile.TileContext(nc) as tc:
        with tc.tile_pool(name="consts", bufs=1) as consts, \
             tc.tile_pool(name="outT_p", bufs=1) as outT_p, \
             tc.tile_pool(name="x2_p", bufs=8) as x2_p:

            ident = consts.tile([128, 128], F32)
            nc.gpsimd.dma_start(out=ident[:], in_=ident_d[:, :])
            qkb = consts.tile([128, 8], F32)
            nc.gpsimd.dma_start(out=qkb[:], in_=qkb_d[:, :])
            bv_bc = consts.tile([128, D], F32)
            nc.gpsimd.dma_start(out=bv_bc[:], in_=bcast_ap(bv_d))
            b2_bc = consts.tile([128, D], F32)
            nc.gpsimd.dma_start(out=b2_bc[:], in_=bcast_ap(b2_d))
            b1c = consts.tile([128, FD // 128], F32)
            nc.gpsimd.dma_start(out=b1c[:], in_=b1c_d[:, :])
            eps_t = consts.tile([128, 1], F32)
            nc.vector.memset(eps_t, EPS)
            ones1 = consts.tile([1, 64], F32)
            nc.vector.memset(ones1, 1.0)

            outT = outT_p.tile([128, 4, TQ], F32)   # concat head outputs ^T

            # =========== stages 1-3 scope ===========
            with tc.tile_pool(name="s13", bufs=1) as s13, \
                 tc.tile_pool(name="xt_p", bufs=3) as xt_p, \
                 tc.tile_pool(name="st_p", bufs=6) as st_p, \
                 tc.tile_pool(name="cn_p", bufs=3) as cn_p, \
                 tc.tile_pool(name="et_p", bufs=4) as et_p, \
                 tc.tile_pool(name="sm_p", bufs=4) as sm_p:

                hnT = s13.tile([128, 4, SK + TQ], F32)
                wq_sb = s13.tile([128, 4, D], F32)
                wk_sb = s13.tile([128, 4, D], F32)
                wv_sb = s13.tile([128, 4, D], F32)
                nc.gpsimd.dma_start(out=wq_sb[:], in_=wq_d[:, :].rearrange("(o p) n -> p o n", p=128))
                nc.gpsimd.dma_start(out=wk_sb[:], in_=wk_d[:, :].rearrange("(o p) n -> p o n", p=128))
                nc.gpsimd.dma_start(out=wv_sb[:], in_=wv_d[:, :].rearrange("(o p) n -> p o n", p=128))

                qT = s13.tile([65, 8, TQ], F32)
                kT = s13.tile([65, 8, SK], F32)
                vA = s13.tile([128, SKT, 8, 65], F32)
                nc.vector.memset(qT[64:65, :, :], 1.0)
                nc.gpsimd.dma_start(out=kT[64:65, :, :], in_=alibi_d[:, :])
                nc.vector.memset(vA[:, :, :, 64:65], 1.0)
                EB = 3  # rotating full-row attn buffers (tail pre-zeroed once)
                E_buf = s13.tile([128, EB, T], F32)
                nc.vector.memset(E_buf[:, :, SK:T], 0.0)

                # ---- stages 1+2 (own psum scope) ----
                with tc.tile_pool(name="ps_t", bufs=2, space="PSUM") as ps_t, \
                     tc.tile_pool(name="ps_qkv", bufs=3, space="PSUM") as ps_qkv:

                    # LN1 + transpose into hnT
                    for i in range(NLN):
                        xt = xt_p.tile([128, D], F32)
                        nc.sync.dma_start(out=xt[:], in_=x_ln[i * 128:(i + 1) * 128, :])
                        stats = st_p.tile([128, 6], F32)
                        nc.vector.bn_stats(out=stats[:], in_=xt[:])
                        mv = st_p.tile([128, 2], F32)
                        nc.vector.bn_aggr(out=mv[:], in_=stats[:])
                        rstd = st_p.tile([128, 1], F32)
                        nc.scalar.activation(out=rstd[:], in_=mv[:, 1:2], func=AF.Sqrt,
                                             bias=eps_t[:], scale=1.0)
                        nc.vector.reciprocal(rstd[:], rstd[:])
                        cn = cn_p.tile([128, D], F32)
                        nc.vector.tensor_scalar(out=cn[:], in0=xt[:], scalar1=mv[:, 0:1],
                                                scalar2=rstd[:], op0=OP.subtract, op1=OP.mult)
                        for j in range(4):
                            pt = ps_t.tile([128, 128], F32)
                            nc.tensor.transpose(pt[:], cn[:, j * 128:(j + 1) * 128], ident[:])
                            nc.scalar.copy(out=hnT[:, j, i * 128:(i + 1) * 128], in_=pt[:])

                    # QKV: head pairs (M=128), then split rows into qT/kT packs
                    for m in range(H // 2):
                        for c in range(TQ // 512):
                            pq = ps_qkv.tile([128, 512], F32, tag="pqk")
                            for j in range(4):
                                nc.tensor.matmul(pq[:], wq_sb[:, j, m * 128:(m + 1) * 128],
                                                 hnT[:, j, SK + c * 512: SK + (c + 1) * 512],
                                                 start=(j == 0), stop=(j == 3))
                            nc.vector.tensor_scalar(out=qT[0:64, 2 * m, c * 512:(c + 1) * 512],
                                                    in0=pq[0:64, :], scalar1=qkb[0:64, m:m + 1],
                                                    scalar2=None, op0=OP.add)
                            nc.vector.tensor_scalar(out=qT[0:64, 2 * m + 1, c * 512:(c + 1) * 512],
                                                    in0=pq[64:128, :], scalar1=qkb[64:128, m:m + 1],
                                                    scalar2=None, op0=OP.add)
                        for c in range(SKC):
                            pk = ps_qkv.tile([128, 512], F32, tag="pqk")
                            for j in range(4):
                                nc.tensor.matmul(pk[:], wk_sb[:, j, m * 128:(m + 1) * 128],
                                                 hnT[:, j, c * 512:(c + 1) * 512],
                                                 start=(j == 0), stop=(j == 3))
                            nc.vector.tensor_scalar(out=kT[0:64, 2 * m, c * 512:(c + 1) * 512],
                                                    in0=pk[0:64, :], scalar1=qkb[0:64, 4 + m:5 + m],
                                                    scalar2=None, op0=OP.add)
                            nc.vector.tensor_scalar(out=kT[0:64, 2 * m + 1, c * 512:(c + 1) * 512],
                                                    in0=pk[64:128, :], scalar1=qkb[64:128, 4 + m:5 + m],
                                                    scalar2=None, op0=OP.add)
                    for i in range(SKT):
                        pv = ps_qkv.tile([128, D], F32, tag="pv")
                        for j in range(4):
                            nc.tensor.matmul(pv[:], hnT[:, j, i * 128:(i + 1) * 128],
                                             wv_sb[:, j, :], start=(j == 0), stop=(j == 3))
                        nc.vector.tensor_tensor(
                            vA[:, i, :, 0:64],
                            pv[:].rearrange("p (h k) -> p h k", h=H),
                            bv_bc[:].rearrange("p (h k) -> p h k", h=H),
                            OP.add)

                # ---- stage 3: attention (own psum scope) ----
                with tc.tile_pool(name="ps_a", bufs=2, space="PSUM") as ps_a, \
                     tc.tile_pool(name="ps_b", bufs=3, space="PSUM") as ps_b, \
                     tc.tile_pool(name="ps_o", bufs=2, space="PSUM") as ps_o, \
                     tc.tile_pool(name="ps_bc", bufs=1, space="PSUM") as ps_bc:
                    for h in range(H):
                        # phase B: out^T_h = v~^T @ exp(scores^T); den lands in row 64
                        for c in range(TQ // 512):
                            po = ps_o.tile([65, 512], F32)
                            for si in range(SKT):
                                pb = ps_b.tile([128, 512], F32)
                                nc.tensor.matmul(pb[:], kT[:, h, si * 128:(si + 1) * 128],
                                                 qT[:, h, c * 512:(c + 1) * 512])
                                et = et_p.tile([128, 512], F32)
                                nc.scalar.activation(out=et[:], in_=pb[:], func=AF.Exp)
                                nc.tensor.matmul(po[:], vA[:, si, h, :], et[:],
                                                 start=(si == 0), stop=(si == SKT - 1))
                            rr = sm_p.tile([1, 512], F32, tag="rr")
                            nc.vector.reciprocal(rr[:], po[64:65, :])
                            pbc = ps_bc.tile([64, 512], F32)
                            nc.tensor.matmul(pbc[:], ones1[:], rr[:])
                            rbc = et_p.tile([64, 512], F32, tag="rbc")
                            nc.vector.tensor_copy(rbc[:], pbc[:])
                            nc.vector.tensor_tensor(
                                outT[(h % 2) * 64:(h % 2) * 64 + 64, h // 2,
                                     c * 512:(c + 1) * 512],
                                po[0:64, :], rbc[:], OP.mult)
                        # phase A: attention-weight rows for the big output
                        for tq in range(TQ // 128):
                            slot = (h * (TQ // 128) + tq) % EB
                            E = E_buf[:, slot, :]
                            den = sm_p.tile([128, 1], F32, tag="den")
                            for c in range(SKC):
                                pa = ps_a.tile([128, 512], F32)
                                nc.tensor.matmul(pa[:], qT[:, h, tq * 128:(tq + 1) * 128],
                                                 kT[:, h, c * 512:(c + 1) * 512])
                                if SKC == 1:
                                    nc.scalar.activation(out=E[:, c * 512:(c + 1) * 512],
                                                         in_=pa[:], func=AF.Exp,
                                                         accum_out=den[:])
                                else:
                                    dpart = sm_p.tile([128, 1], F32, tag="dpart")
                                    nc.scalar.activation(out=E[:, c * 512:(c + 1) * 512],
                                                         in_=pa[:], func=AF.Exp,
                                                         accum_out=dpart[:])
                                    if c == 0:
                                        nc.vector.tensor_copy(den[:], dpart[:])
                                    else:
                                        nc.vector.tensor_add(out=den[:], in0=den[:],
                                                             in1=dpart[:])
                            r = sm_p.tile([128, 1], F32, tag="r")
                            nc.vector.reciprocal(r[:], den[:])
                            nc.vector.tensor_scalar_mul(E[:, 0:SK], E[:, 0:SK], r[:])
                            eng = nc.sync if (tq % 2 == 0) else nc.scalar
                            eng.dma_start(out=attn_o[h, tq * 128:(tq + 1) * 128, :],
                                          in_=E[:])

            # =========== stages 4-5 scope ===========
            with tc.tile_pool(name="s45", bufs=1) as s45, \
                 tc.tile_pool(name="xr_p", bufs=3) as xr_p, \
                 tc.tile_pool(name="st2_p", bufs=6) as st2_p, \
                 tc.tile_pool(name="cn2_p", bufs=3) as cn2_p, \
                 tc.tile_pool(name="f1_p", bufs=1) as f1_p, \
                 tc.tile_pool(name="xo_p", bufs=3) as xo_p, \
                 tc.tile_pool(name="ps_p", bufs=2, space="PSUM") as ps_p, \
                 tc.tile_pool(name="ps_t2", bufs=2, space="PSUM") as ps_t2, \
                 tc.tile_pool(name="ps_f", bufs=2, space="PSUM") as ps_f, \
                 tc.tile_pool(name="ps_2", bufs=2, space="PSUM") as ps_2:

                wp_sb = s45.tile([128, 4, D], F32)
                nc.gpsimd.dma_start(out=wp_sb[:], in_=wp_d[:, :].rearrange("(o p) n -> p o n", p=128))
                w1_sb = s45.tile([128, 4, FD], F32)
                nc.gpsimd.dma_start(out=w1_sb[:], in_=w1_d[:, :].rearrange("(o p) n -> p o n", p=128))
                w2_sb = s45.tile([128, 16, D], F32)
                nc.gpsimd.dma_start(out=w2_sb[:], in_=w2_d[:, :].rearrange("(o p) n -> p o n", p=128))
                h2T = s45.tile([128, 4, TQ], F32)
                x2s = []

                # ---- stage 4: proj + residual + LN2 (+ transpose) ----
                for tq in range(TQ // 128):
                    pp = ps_p.tile([128, D], F32)
                    for j in range(4):
                        nc.tensor.matmul(pp[:], outT[:, j, tq * 128:(tq + 1) * 128],
                                         wp_sb[:, j, :], start=(j == 0), stop=(j == 3))
                    xr = xr_p.tile([128, D], F32)
                    nc.sync.dma_start(out=xr[:], in_=x_res[tq * 128:(tq + 1) * 128, :])
                    x2 = x2_p.tile([128, D], F32, tag="x2")
                    nc.vector.tensor_add(out=x2[:], in0=pp[:], in1=xr[:])
                    x2s.append(x2)
                    stats = st2_p.tile([128, 6], F32)
                    nc.vector.bn_stats(out=stats[:], in_=x2[:])
                    mv = st2_p.tile([128, 2], F32)
                    nc.vector.bn_aggr(out=mv[:], in_=stats[:])
                    rstd = st2_p.tile([128, 1], F32)
                    nc.scalar.activation(out=rstd[:], in_=mv[:, 1:2], func=AF.Sqrt,
                                         bias=eps_t[:], scale=1.0)
                    nc.vector.reciprocal(rstd[:], rstd[:])
                    cn2 = cn2_p.tile([128, D], F32)
                    nc.vector.tensor_scalar(out=cn2[:], in0=x2[:], scalar1=mv[:, 0:1],
                                            scalar2=rstd[:], op0=OP.subtract, op1=OP.mult)
                    for j in range(4):
                        pt = ps_t2.tile([128, 128], F32)
                        nc.tensor.transpose(pt[:], cn2[:, j * 128:(j + 1) * 128], ident[:])
                        nc.scalar.copy(out=h2T[:, j, tq * 128:(tq + 1) * 128], in_=pt[:])

                # ---- stage 5: FFN ----
                for c in range(TQ // 512):
                    f1 = f1_p.tile([128, 16, 512], F32)
                    for m in range(16):
                        pf = ps_f.tile([128, 512], F32)
                        for j in range(4):
                            nc.tensor.matmul(pf[:], w1_sb[:, j, m * 128:(m + 1) * 128],
                                             h2T[:, j, c * 512:(c + 1) * 512],
                                             start=(j == 0), stop=(j == 3))
                        nc.scalar.activation(out=f1[:, m, :], in_=pf[:], func=AF.Relu,
                                             bias=b1c[:, m:m + 1], scale=1.0)
                    for u in range(4):
                        tq = c * 4 + u
                        p2 = ps_2.tile([128, D], F32)
                        for m in range(16):
                            nc.tensor.matmul(p2[:], f1[:, m, u * 128:(u + 1) * 128],
                                             w2_sb[:, m, :], start=(m == 0), stop=(m == 15))
                        xo = xo_p.tile([128, D], F32)
                        nc.vector.tensor_add(out=xo[:], in0=p2[:], in1=x2s[tq][:])
                        nc.vector.tensor_add(out=xo[:], in0=xo[:], in1=b2_bc[:])
                        nc.sync.dma_start(out=xout_o[tq * 128:(tq + 1) * 128, :], in_=xo[:])

    nc.finalize()
    return nc


def _host_prep(x, Wq, Wk, Wv, Wp, bp, W1, b1, W2, b2, g1, be1, g2, be2):
    """Fold LN affine params into projection weights (exact algebra) and
    build per-core input maps."""
    f32 = np.float32
    x = np.asarray(x, f32)
    Wq = np.asarray(Wq, f32); Wk = np.asarray(Wk, f32); Wv = np.asarray(Wv, f32)
    Wp = np.ascontiguousarray(np.asarray(Wp, f32))
    W1 = np.asarray(W1, f32); W2 = np.ascontiguousarray(np.asarray(W2, f32))
    bp = np.asarray(bp, f32); b1 = np.asarray(b1, f32); b2 = np.asarray(b2, f32)
    g1 = np.asarray(g1, f32); be1 = np.asarray(be1, f32)
    g2 = np.asarray(g2, f32); be2 = np.asarray(be2, f32)

    scale = 1.0 / np.sqrt(HS)
    wq_f = np.ascontiguousarray(np.transpose(Wq, (1, 0, 2)).reshape(D, D)
                                * g1[:, None] * scale).astype(f32)
    wk_f = np.ascontiguousarray(np.transpose(Wk, (1, 0, 2)).reshape(D, D)
                                * g1[:, None]).astype(f32)
    wv_f = np.ascontiguousarray(np.transpose(Wv, (1, 0, 2)).reshape(D, D)
                                * g1[:, None]).astype(f32)
    bq = (np.einsum('d,hdk->hk', be1, Wq) * scale).astype(f32)   # [H, HS]
    bk = np.einsum('d,hdk->hk', be1, Wk).astype(f32)
    bv = np.einsum('d,hdk->hk', be1, Wv).reshape(D).astype(f32)
    # cols 0-3: q head-pairs (head 2m on partitions 0:64, 2m+1 on 64:128);
    # cols 4-7: same for k
    qkbias = np.zeros((128, 8), f32)
    for m in range(H // 2):
        qkbias[0:64, m] = bq[2 * m]
        qkbias[64:128, m] = bq[2 * m + 1]
        qkbias[0:64, 4 + m] = bk[2 * m]
        qkbias[64:128, 4 + m] = bk[2 * m + 1]

    w1_f = np.ascontiguousarray(W1 * g2[:, None]).astype(f32)
    b1_f = (b1 + be2 @ W1).astype(f32)
    b1col = np.ascontiguousarray(b1_f.reshape(FD // 128, 128).T).astype(f32)

    slopes = (2.0 ** (-8.0 / H)) * np.arange(1, H + 1, dtype=f32)

    # ---- choose SK: reference attn provably exact-zero for s >= SK ----
    sq = np.array([np.linalg.norm(wq_f[:, h * HS:(h + 1) * HS], 2) for h in range(H)])
    sk_ = np.array([np.linalg.norm(wk_f[:, h * HS:(h + 1) * HS], 2) for h in range(H)])
    Bq = np.sqrt(D) * sq + np.linalg.norm(bq, axis=1)
    Bk = np.sqrt(D) * sk_ + np.linalg.norm(bk, axis=1)
    Bh = Bq * Bk      # |scores| <= Bh per head (||cn_row|| <= sqrt(D))
    SK = None
    for cand in (512, 1024):
        if np.all(slopes * cand - 2 * Bh > 105.0):
            SK = cand
            break
    if SK is None:
        raise RuntimeError(f"score bound too large for truncation: Bh={Bh}")

    alibi = np.ascontiguousarray(
        -slopes[:, None] * np.arange(SK, dtype=f32)[None, :]).astype(f32)

    shared = {
        "wq": wq_f, "wk": wk_f, "wv": wv_f, "wp": Wp,
        "w1": w1_f, "w2": W2,
        "qkbias": qkbias, "bv_row": bv.reshape(1, D),
        "b1col": b1col, "b2_row": np.ascontiguousarray(b2.reshape(1, D)),
        "alibi": alibi, "ident": np.eye(128, dtype=f32),
    }
    xbp = x + bp[None, None, :]
    in_maps = []
    for c in range(NC):
        b, th = c // 2, c % 2
        t0 = th * TQ
        m = dict(shared)
        m["x_ln"] = np.ascontiguousarray(
            np.concatenate([x[b, 0:SK], x[b, t0:t0 + TQ]], axis=0))
        m["x_res"] = np.ascontiguousarray(xbp[b, t0:t0 + TQ])
        in_maps.append(m)
    return SK, in_maps


def kernel(x, Wq, Wk, Wv, Wp, bp, W1, b1, W2, b2, g1, be1, g2, be2):
    from concourse.bass_utils import run_bass_kernel_spmd

    SK, in_maps = _host_prep(x, Wq, Wk, Wv, Wp, bp, W1, b1, W2, b2,
                             g1, be1, g2, be2)
    key = (SK, True)
    if key not in _BUILT:
        _BUILT[key] = _build(SK, explicit_zeros=True)
    nc = _BUILT[key]

    res = run_bass_kernel_spmd(nc, in_maps, core_ids=list(range(NC)))

    x_out = np.empty((B, T, D), np.float32)
    attn = np.empty((H, B, T, T), np.float32)
    for c in range(NC):
        b, th = c // 2, c % 2
        t0 = th * TQ
        x_out[b, t0:t0 + TQ] = res.results[c]["xout"]
        attn[:, b, t0:t0 + TQ, :] = res.results[c]["attn"]
    return (x_out, attn)


# revision 11
# speedup vs baseline: 1.8499x; 1.5749x over previous
"""Trainium2 Bass kernel for nn_BlockALiBi (dense transformer block with ALiBi
attention that also returns the full attention-weight tensor).

Reference computation (B=4, T=2048, D=512, H=8, HS=64):
    hn = LN(x; g1, be1)
    q,k,v = per-head projections of hn (no bias)
    scores = q @ k^T / sqrt(HS) + alibi  (alibi[h, s] = -slope_h * s)
    attn = softmax(scores, axis=-1)
    x = x + (concat_heads(attn @ v)) @ Wp + bp
    x = x + relu(LN(x; g2, be2) @ W1 + b1) @ W2 + b2
    returns (x, attn transposed to [H,B,T,T])

Sharding: 8 cores; core c owns (batch b = c//2, query half th = c%2, 1024
query rows).  Each core computes the block for its query rows and emits its
[H, 1024, T] slice of the attention weights.  No collectives.

Key algebraic fact exploited: the ALiBi slopes are 0.5*(h+1) and |scores| is
provably bounded (Cauchy-Schwarz with spectral norms computed host-side), so
for key positions s >= SK (512 here) the reference softmax's exp(x - max) has
argument < -104 and underflows to an exact 0.0 in fp32.  The kernel computes
attention over the first SK keys only and writes exact zeros elsewhere; the
bound is validated at runtime against the actual weights (falls back to a
larger SK if ever violated).
"""

import numpy as np

B, T, D, H = 4, 2048, 512, 8
HS = D // H           # 64
FD = 4 * D            # 2048 ffn hidden
NC = 8                # cores
TQ = T // 2           # query rows per core
EPS = 1e-5

USE_F32R = True       # single-pass fp32 matmuls (4x PE rate, ~1e-4 rel err)
_BUILT = {}           # (SK, use_f32r) -> finalized Bass object


def _build(SK, explicit_zeros=True, use_f32r=False):
    """Build and finalize the per-core Bass kernel for SK kept key positions.

    use_f32r: run matmuls in single-pass fp32 (float32r, ~4x faster PE) --
    operands are rounded to the fp32r grid by their producing instruction.
    """
    import concourse.bass as bass
    import concourse.tile as tile
    from concourse import bacc, mybir

    F32 = mybir.dt.float32
    FMM = mybir.dt.float32r if use_f32r else mybir.dt.float32
    AF = mybir.ActivationFunctionType
    OP = mybir.AluOpType

    SKT = SK // 128           # s-tiles of 128 (phase B / v)
    SKC = SK // 512           # s-chunks of 512 (phase A scores)
    NLN = (SK + TQ) // 128    # LN1 token tiles (keys then queries)
    ZW = T - SK               # zero-filled tail width per attn row

    nc = bacc.Bacc("TRN2", target_bir_lowering=False)

    # ---- I/O ----
    x_ln = nc.dram_tensor("x_ln", [SK + TQ, D], F32, kind="ExternalInput")
    x_res = nc.dram_tensor("x_res", [TQ, D], F32, kind="ExternalInput")
    wq_d = nc.dram_tensor("wq", [D, D], F32, kind="ExternalInput")
    wk_d = nc.dram_tensor("wk", [D, D], F32, kind="ExternalInput")
    wv_d = nc.dram_tensor("wv", [D, D], F32, kind="ExternalInput")
    wp_d = nc.dram_tensor("wp", [D, D], F32, kind="ExternalInput")
    w1_d = nc.dram_tensor("w1", [D, FD], F32, kind="ExternalInput")
    w2_d = nc.dram_tensor("w2", [FD, D], F32, kind="ExternalInput")
    qkb_d = nc.dram_tensor("qkbias", [128, 8], F32, kind="ExternalInput")
    bv_d = nc.dram_tensor("bv_row", [1, D], F32, kind="ExternalInput")
    b1c_d = nc.dram_tensor("b1col", [128, FD // 128], F32, kind="ExternalInput")
    b2_d = nc.dram_tensor("b2_row", [1, D], F32, kind="ExternalInput")
    alibi_d = nc.dram_tensor("alibi", [H, SK], F32, kind="ExternalInput")
    ident_d = nc.dram_tensor("ident", [128, 128], F32, kind="ExternalInput")

    attn_o = nc.dram_tensor("attn", [H, TQ, T], F32, kind="ExternalOutput")
    xout_o = nc.dram_tensor("xout", [TQ, D], F32, kind="ExternalOutput")

    def bcast_ap(dram_handle, p=128):
        # partition-replicating DMA source (step 0 over partitions)
        ap = dram_handle[:, :]
        return bass.AP(tensor=ap.tensor, offset=ap.offset,
                       ap=[[0, p]] + list(ap.ap))

    with tile.TileContext(nc) as tc:
        with tc.tile_pool(name="consts", bufs=1) as consts, \
             tc.tile_pool(name="outT_p", bufs=1) as outT_p, \
             tc.tile_pool(name="x2_p", bufs=8) as x2_p:

            ident = consts.tile([128, 128], F32)
            nc.gpsimd.dma_start(out=ident[:], in_=ident_d[:, :])
            qkb = consts.tile([128, 8], F32)
            nc.gpsimd.dma_start(out=qkb[:], in_=qkb_d[:, :])
            bv_bc = consts.tile([128, D], F32)
            nc.gpsimd.dma_start(out=bv_bc[:], in_=bcast_ap(bv_d))
            b2_bc = consts.tile([128, D], F32)
            nc.gpsimd.dma_start(out=b2_bc[:], in_=bcast_ap(b2_d))
            b1c = consts.tile([128, FD // 128], F32)
            nc.gpsimd.dma_start(out=b1c[:], in_=b1c_d[:, :])
            eps_t = consts.tile([128, 1], F32)
            nc.vector.memset(eps_t, EPS)
            ones1 = consts.tile([1, 64], F32)
            nc.vector.memset(ones1, 1.0)

            outT = outT_p.tile([128, 4, TQ], FMM)   # concat head outputs ^T

            # =========== stages 1-3 scope ===========
            with tc.tile_pool(name="s13", bufs=1) as s13, \
                 tc.tile_pool(name="xt_p", bufs=3) as xt_p, \
                 tc.tile_pool(name="st_p", bufs=6) as st_p, \
                 tc.tile_pool(name="cn_p", bufs=3) as cn_p, \
                 tc.tile_pool(name="et_p", bufs=4) as et_p, \
                 tc.tile_pool(name="sm_p", bufs=4) as sm_p:

                hnT = s13.tile([128, 4, SK + TQ], FMM)
                wq_sb = s13.tile([128, 4, D], FMM)
                wk_sb = s13.tile([128, 4, D], FMM)
                wv_sb = s13.tile([128, 4, D], FMM)
                nc.gpsimd.dma_start(out=wq_sb[:], in_=wq_d[:, :].rearrange("(o p) n -> p o n", p=128))
                nc.gpsimd.dma_start(out=wk_sb[:], in_=wk_d[:, :].rearrange("(o p) n -> p o n", p=128))
                nc.gpsimd.dma_start(out=wv_sb[:], in_=wv_d[:, :].rearrange("(o p) n -> p o n", p=128))

                qT = s13.tile([65, 8, TQ], FMM)
                kT = s13.tile([65, 8, SK], FMM)
                vA = s13.tile([128, SKT, 8, 65], FMM)
                if use_f32r:
                    ones_sc = s13.tile([128, TQ], F32)
                    nc.vector.memset(ones_sc[:], 1.0)
                    os_ap = ones_sc[:]
                    nc.vector.tensor_copy(
                        qT[64:65, :, :],
                        bass.AP(tensor=os_ap.tensor, offset=os_ap.offset,
                                ap=[list(os_ap.ap[0][:1]) + [1], [0, 8], [1, TQ]]))
                    nc.vector.tensor_copy(
                        vA[:, :, :, 64:65],
                        bass.AP(tensor=os_ap.tensor, offset=os_ap.offset,
                                ap=[list(os_ap.ap[0][:1]) + [128], [0, SKT], [0, 8], [0, 1]]))
                else:
                    nc.vector.memset(qT[64:65, :, :], 1.0)
                    nc.vector.memset(vA[:, :, :, 64:65], 1.0)
                nc.gpsimd.dma_start(out=kT[64:65, :, :], in_=alibi_d[:, :])
                EB = 3  # rotating full-row attn buffers (tail pre-zeroed once)
                E_buf = s13.tile([128, EB, T], F32)
                nc.vector.memset(E_buf[:, :, SK:T], 0.0)

                # ---- stages 1+2 (own psum scope) ----
                with tc.tile_pool(name="ps_t", bufs=2, space="PSUM") as ps_t, \
                     tc.tile_pool(name="ps_qkv", bufs=3, space="PSUM") as ps_qkv:

                    # LN1 + transpose into hnT
                    for i in range(NLN):
                        xt = xt_p.tile([128, D], F32)
                        nc.sync.dma_start(out=xt[:], in_=x_ln[i * 128:(i + 1) * 128, :])
                        stats = st_p.tile([128, 6], F32)
                        nc.vector.bn_stats(out=stats[:], in_=xt[:])
                        mv = st_p.tile([128, 2], F32)
                        nc.vector.bn_aggr(out=mv[:], in_=stats[:])
                        rstd = st_p.tile([128, 1], F32)
                        nc.scalar.activation(out=rstd[:], in_=mv[:, 1:2], func=AF.Sqrt,
                                             bias=eps_t[:], scale=1.0)
                        nc.vector.reciprocal(rstd[:], rstd[:])
                        cn = cn_p.tile([128, D], F32)
                        nc.vector.tensor_scalar(out=cn[:], in0=xt[:], scalar1=mv[:, 0:1],
                                                scalar2=rstd[:], op0=OP.subtract, op1=OP.mult)
                        for j in range(4):
                            pt = ps_t.tile([128, 128], F32)
                            nc.tensor.transpose(pt[:], cn[:, j * 128:(j + 1) * 128], ident[:])
                            nc.scalar.copy(out=hnT[:, j, i * 128:(i + 1) * 128], in_=pt[:])

                    # QKV: head pairs (M=128), then split rows into qT/kT packs
                    for m in range(H // 2):
                        for c in range(TQ // 512):
                            pq = ps_qkv.tile([128, 512], F32, tag="pqk")
                            for j in range(4):
                                nc.tensor.matmul(pq[:], wq_sb[:, j, m * 128:(m + 1) * 128],
                                                 hnT[:, j, SK + c * 512: SK + (c + 1) * 512],
                                                 start=(j == 0), stop=(j == 3))
                            nc.vector.tensor_scalar(out=qT[0:64, 2 * m, c * 512:(c + 1) * 512],
                                                    in0=pq[0:64, :], scalar1=qkb[0:64, m:m + 1],
                                                    scalar2=None, op0=OP.add)
                            nc.vector.tensor_scalar(out=qT[0:64, 2 * m + 1, c * 512:(c + 1) * 512],
                                                    in0=pq[64:128, :], scalar1=qkb[64:128, m:m + 1],
                                                    scalar2=None, op0=OP.add)
                        for c in range(SKC):
                            pk = ps_qkv.tile([128, 512], F32, tag="pqk")
                            for j in range(4):
                                nc.tensor.matmul(pk[:], wk_sb[:, j, m * 128:(m + 1) * 128],
                                                 hnT[:, j, c * 512:(c + 1) * 512],
                                                 start=(j == 0), stop=(j == 3))
                            nc.vector.tensor_scalar(out=kT[0:64, 2 * m, c * 512:(c + 1) * 512],
                                                    in0=pk[0:64, :], scalar1=qkb[0:64, 4 + m:5 + m],
                                                    scalar2=None, op0=OP.add)
                            nc.vector.tensor_scalar(out=kT[0:64, 2 * m + 1, c * 512:(c + 1) * 512],
                                                    in0=pk[64:128, :], scalar1=qkb[64:128, 4 + m:5 + m],
                                                    scalar2=None, op0=OP.add)
                    for i in range(SKT):
                        pv = ps_qkv.tile([128, D], F32, tag="pv")
                        for j in range(4):
                            nc.tensor.matmul(pv[:], hnT[:, j, i * 128:(i + 1) * 128],
                                             wv_sb[:, j, :], start=(j == 0), stop=(j == 3))
                        nc.vector.tensor_tensor(
                            vA[:, i, :, 0:64],
                            pv[:].rearrange("p (h k) -> p h k", h=H),
                            bv_bc[:].rearrange("p (h k) -> p h k", h=H),
                            OP.add)

                # ---- stage 3: attention (own psum scope) ----
                with tc.tile_pool(name="ps_a", bufs=2, space="PSUM") as ps_a, \
                     tc.tile_pool(name="ps_b", bufs=3, space="PSUM") as ps_b, \
                     tc.tile_pool(name="ps_o", bufs=2, space="PSUM") as ps_o, \
                     tc.tile_pool(name="ps_bc", bufs=1, space="PSUM") as ps_bc:
                    for h in range(H):
                        # phase B: out^T_h = v~^T @ exp(scores^T); den lands in row 64
                        for c in range(TQ // 512):
                            po = ps_o.tile([65, 512], F32)
                            for si in range(SKT):
                                pb = ps_b.tile([128, 512], F32)
                                nc.tensor.matmul(pb[:], kT[:, h, si * 128:(si + 1) * 128],
                                                 qT[:, h, c * 512:(c + 1) * 512])
                                et = et_p.tile([128, 512], FMM)
                                nc.scalar.activation(out=et[:], in_=pb[:], func=AF.Exp)
                                nc.tensor.matmul(po[:], vA[:, si, h, :], et[:],
                                                 start=(si == 0), stop=(si == SKT - 1))
                            rr = sm_p.tile([1, 512], F32, tag="rr")
                            nc.vector.reciprocal(rr[:], po[64:65, :])
                            pbc = ps_bc.tile([64, 512], F32)
                            nc.tensor.matmul(pbc[:], ones1[:], rr[:])
                            rbc = et_p.tile([64, 512], F32, tag="rbc")
                            nc.vector.tensor_copy(rbc[:], pbc[:])
                            nc.vector.tensor_tensor(
                                outT[(h % 2) * 64:(h % 2) * 64 + 64, h // 2,
                                     c * 512:(c + 1) * 512],
                                po[0:64, :], rbc[:], OP.mult)
                        # phase A: attention-weight rows for the big output
                        for tq in range(TQ // 128):
                            slot = (h * (TQ // 128) + tq) % EB
                            E = E_buf[:, slot, :]
                            den = sm_p.tile([128, 1], F32, tag="den")
                            for c in range(SKC):
                                pa = ps_a.tile([128, 512], F32)
                                nc.tensor.matmul(pa[:], qT[:, h, tq * 128:(tq + 1) * 128],
                                                 kT[:, h, c * 512:(c + 1) * 512])
                                if SKC == 1:
                                    nc.scalar.activation(out=E[:, c * 512:(c + 1) * 512],
                                                         in_=pa[:], func=AF.Exp,
                                                         accum_out=den[:])
                                else:
                                    dpart = sm_p.tile([128, 1], F32, tag="dpart")
                                    nc.scalar.activation(out=E[:, c * 512:(c + 1) * 512],
                                                         in_=pa[:], func=AF.Exp,
                                                         accum_out=dpart[:])
                                    if c == 0:
                                        nc.vector.tensor_copy(den[:], dpart[:])
                                    else:
                                        nc.vector.tensor_add(out=den[:], in0=den[:],
                                                             in1=dpart[:])
                            r = sm_p.tile([128, 1], F32, tag="r")
                            nc.vector.reciprocal(r[:], den[:])
                            nc.vector.tensor_scalar_mul(E[:, 0:SK], E[:, 0:SK], r[:])
                            eng = nc.sync if (tq % 2 == 0) else nc.scalar
                            eng.dma_start(out=attn_o[h, tq * 128:(tq + 1) * 128, :],
                                          in_=E[:])

            # =========== stages 4-5 scope ===========
            with tc.tile_pool(name="s45", bufs=1) as s45, \
                 tc.tile_pool(name="xr_p", bufs=3) as xr_p, \
                 tc.tile_pool(name="st2_p", bufs=6) as st2_p, \
                 tc.tile_pool(name="cn2_p", bufs=3) as cn2_p, \
                 tc.tile_pool(name="f1_p", bufs=1) as f1_p, \
                 tc.tile_pool(name="xo_p", bufs=3) as xo_p, \
                 tc.tile_pool(name="ps_p", bufs=2, space="PSUM") as ps_p, \
                 tc.tile_pool(name="ps_t2", bufs=2, space="PSUM") as ps_t2, \
                 tc.tile_pool(name="ps_f", bufs=2, space="PSUM") as ps_f, \
                 tc.tile_pool(name="ps_2", bufs=2, space="PSUM") as ps_2:

                wp_sb = s45.tile([128, 4, D], FMM)
                nc.gpsimd.dma_start(out=wp_sb[:], in_=wp_d[:, :].rearrange("(o p) n -> p o n", p=128))
                w1_sb = s45.tile([128, 4, FD], FMM)
                nc.gpsimd.dma_start(out=w1_sb[:], in_=w1_d[:, :].rearrange("(o p) n -> p o n", p=128))
                w2_sb = s45.tile([128, 16, D], FMM)
                nc.gpsimd.dma_start(out=w2_sb[:], in_=w2_d[:, :].rearrange("(o p) n -> p o n", p=128))
                h2T = s45.tile([128, 4, TQ], FMM)
                x2s = []

                # ---- stage 4: proj + residual + LN2 (+ transpose) ----
                for tq in range(TQ // 128):
                    pp = ps_p.tile([128, D], F32)
                    for j in range(4):
                        nc.tensor.matmul(pp[:], outT[:, j, tq * 128:(tq + 1) * 128],
                                         wp_sb[:, j, :], start=(j == 0), stop=(j == 3))
                    xr = xr_p.tile([128, D], F32)
                    nc.sync.dma_start(out=xr[:], in_=x_res[tq * 128:(tq + 1) * 128, :])
                    x2 = x2_p.tile([128, D], F32, tag="x2")
                    nc.vector.tensor_add(out=x2[:], in0=pp[:], in1=xr[:])
                    x2s.append(x2)
                    stats = st2_p.tile([128, 6], F32)
                    nc.vector.bn_stats(out=stats[:], in_=x2[:])
                    mv = st2_p.tile([128, 2], F32)
                    nc.vector.bn_aggr(out=mv[:], in_=stats[:])
                    rstd = st2_p.tile([128, 1], F32)
                    nc.scalar.activation(out=rstd[:], in_=mv[:, 1:2], func=AF.Sqrt,
                                         bias=eps_t[:], scale=1.0)
                    nc.vector.reciprocal(rstd[:], rstd[:])
                    cn2 = cn2_p.tile([128, D], F32)
                    nc.vector.tensor_scalar(out=cn2[:], in0=x2[:], scalar1=mv[:, 0:1],
                                            scalar2=rstd[:], op0=OP.subtract, op1=OP.mult)
                    for j in range(4):
                        pt = ps_t2.tile([128, 128], F32)
                        nc.tensor.transpose(pt[:], cn2[:, j * 128:(j + 1) * 128], ident[:])
                        nc.scalar.copy(out=h2T[:, j, tq * 128:(tq + 1) * 128], in_=pt[:])

                # ---- stage 5: FFN ----
                for c in range(TQ // 512):
                    f1 = f1_p.tile([128, 16, 512], F32)
                    for m in range(16):
                        pf = ps_f.tile([128, 512], F32)
                        for j in range(4):
                            nc.tensor.matmul(pf[:], w1_sb[:, j, m * 128:(m + 1) * 128],
                                             h2T[:, j, c * 512:(c + 1) * 512],
                                             start=(j == 0), stop=(j == 3))
                        nc.scalar.activation(out=f1[:, m, :], in_=pf[:], func=AF.Relu,
                                             bias=b1c[:, m:m + 1], scale=1.0)
                    for u in range(4):
                        tq = c * 4 + u
                        p2 = ps_2.tile([128, D], F32)
                        for m in range(16):
                            nc.tensor.matmul(p2[:], f1[:, m, u * 128:(u + 1) * 128],
                                             w2_sb[:, m, :], start=(m == 0), stop=(m == 15))
                        xo = xo_p.tile([128, D], F32)
                        nc.vector.tensor_add(out=xo[:], in0=p2[:], in1=x2s[tq][:])
                        nc.vector.tensor_add(out=xo[:], in0=xo[:], in1=b2_bc[:])
                        nc.sync.dma_start(out=xout_o[tq * 128:(tq + 1) * 128, :], in_=xo[:])

    nc.finalize()
    return nc


def _host_prep(x, Wq, Wk, Wv, Wp, bp, W1, b1, W2, b2, g1, be1, g2, be2):
    """Fold LN affine params into projection weights (exact algebra) and
    build per-core input maps."""
    f32 = np.float32
    x = np.asarray(x, f32)
    Wq = np.asarray(Wq, f32); Wk = np.asarray(Wk, f32); Wv = np.asarray(Wv, f32)
    Wp = np.ascontiguousarray(np.asarray(Wp, f32))
    W1 = np.asarray(W1, f32); W2 = np.ascontiguousarray(np.asarray(W2, f32))
    bp = np.asarray(bp, f32); b1 = np.asarray(b1, f32); b2 = np.asarray(b2, f32)
    g1 = np.asarray(g1, f32); be1 = np.asarray(be1, f32)
    g2 = np.asarray(g2, f32); be2 = np.asarray(be2, f32)

    scale = 1.0 / np.sqrt(HS)
    wq_f = np.ascontiguousarray(np.transpose(Wq, (1, 0, 2)).reshape(D, D)
                                * g1[:, None] * scale).astype(f32)
    wk_f = np.ascontiguousarray(np.transpose(Wk, (1, 0, 2)).reshape(D, D)
                                * g1[:, None]).astype(f32)
    wv_f = np.ascontiguousarray(np.transpose(Wv, (1, 0, 2)).reshape(D, D)
                                * g1[:, None]).astype(f32)
    bq = (np.einsum('d,hdk->hk', be1, Wq) * scale).astype(f32)   # [H, HS]
    bk = np.einsum('d,hdk->hk', be1, Wk).astype(f32)
    bv = np.einsum('d,hdk->hk', be1, Wv).reshape(D).astype(f32)
    # cols 0-3: q head-pairs (head 2m on partitions 0:64, 2m+1 on 64:128);
    # cols 4-7: same for k
    qkbias = np.zeros((128, 8), f32)
    for m in range(H // 2):
        qkbias[0:64, m] = bq[2 * m]
        qkbias[64:128, m] = bq[2 * m + 1]
        qkbias[0:64, 4 + m] = bk[2 * m]
        qkbias[64:128, 4 + m] = bk[2 * m + 1]

    w1_f = np.ascontiguousarray(W1 * g2[:, None]).astype(f32)
    b1_f = (b1 + be2 @ W1).astype(f32)
    b1col = np.ascontiguousarray(b1_f.reshape(FD // 128, 128).T).astype(f32)

    slopes = (2.0 ** (-8.0 / H)) * np.arange(1, H + 1, dtype=f32)

    # ---- choose SK: reference attn provably exact-zero for s >= SK ----
    sq = np.array([np.linalg.norm(wq_f[:, h * HS:(h + 1) * HS], 2) for h in range(H)])
    sk_ = np.array([np.linalg.norm(wk_f[:, h * HS:(h + 1) * HS], 2) for h in range(H)])
    Bq = np.sqrt(D) * sq + np.linalg.norm(bq, axis=1)
    Bk = np.sqrt(D) * sk_ + np.linalg.norm(bk, axis=1)
    Bh = Bq * Bk      # |scores| <= Bh per head (||cn_row|| <= sqrt(D))
    SK = None
    for cand in (512, 1024):
        if np.all(slopes * cand - 2 * Bh > 105.0):
            SK = cand
            break
    if SK is None:
        raise RuntimeError(f"score bound too large for truncation: Bh={Bh}")

    alibi = np.ascontiguousarray(
        -slopes[:, None] * np.arange(SK, dtype=f32)[None, :]).astype(f32)

    shared = {
        "wq": wq_f, "wk": wk_f, "wv": wv_f, "wp": Wp,
        "w1": w1_f, "w2": W2,
        "qkbias": qkbias, "bv_row": bv.reshape(1, D),
        "b1col": b1col, "b2_row": np.ascontiguousarray(b2.reshape(1, D)),
        "alibi": alibi, "ident": np.eye(128, dtype=f32),
    }
    xbp = x + bp[None, None, :]
    in_maps = []
    for c in range(NC):
        b, th = c // 2, c % 2
        t0 = th * TQ
        m = dict(shared)
        m["x_ln"] = np.ascontiguousarray(
            np.concatenate([x[b, 0:SK], x[b, t0:t0 + TQ]], axis=0))
        m["x_res"] = np.ascontiguousarray(xbp[b, t0:t0 + TQ])
        in_maps.append(m)
    return SK, in_maps


def kernel(x, Wq, Wk, Wv, Wp, bp, W1, b1, W2, b2, g1, be1, g2, be2):
    from concourse.bass_utils import run_bass_kernel_spmd

    SK, in_maps = _host_prep(x, Wq, Wk, Wv, Wp, bp, W1, b1, W2, b2,
                             g1, be1, g2, be2)
    key = (SK, USE_F32R)
    if key not in _BUILT:
        _BUILT[key] = _build(SK, use_f32r=USE_F32R)
    nc = _BUILT[key]

    res = run_bass_kernel_spmd(nc, in_maps, core_ids=list(range(NC)))

    x_out = np.empty((B, T, D), np.float32)
    attn = np.empty((H, B, T, T), np.float32)
    for c in range(NC):
        b, th = c // 2, c % 2
        t0 = th * TQ
        x_out[b, t0:t0 + TQ] = res.results[c]["xout"]
        attn[:, b, t0:t0 + TQ, :] = res.results[c]["attn"]
    return (x_out, attn)
